# revision 7
# baseline (speedup 1.0000x reference)
"""Trainium2 Bass kernel for nn_Attention_4063039062503.

Reference (per batch b, C=128 channels, N=4096 points):
    q = W1 @ x + b1;  k = W2 @ x + b2          # [C, N]
    s[n, m] = q[:, n] . k[:, m]                # [N, N]
    a = softmax(s, axis=m)
    out = relu(x + x @ a.T)                    # out[:, n] = x @ a[n, :]

Sharding: 8 cores, core i -> batch i//2, query half i%2 (2048 queries),
full 4096 keys local (no collectives).

Per-core plan (flash-attention style, scores never hit DRAM):
  - projections on TensorE in fp32r (measured ~1.6e-4 rel err, full speed)
  - S^T tiles [m=128, q=512] in PSUM via fp32r matmuls (K tile stationary)
  - exp on ScalarE PSUM->SBUF, bf16 out, groups of 3 m-tiles per call
  - O[c, q] += xT[m-tile] @ E on TensorE in bf16, accumulated in one PSUM bank
  - row-sums: DVE bf16 accumulate across groups + one ones-matmul
  - normalize via fp32 reciprocal + fp32 broadcast-matmul, then
    residual add + relu on DVE, DMA out
"""
import os
from contextlib import ExitStack

import numpy as np
import ml_dtypes

import concourse.bass as bass
import concourse.tile as tile
from concourse import bacc, mybir
from concourse.bass_utils import run_bass_kernel_spmd

B = 4
C = 128
N = 4096            # keys per batch
NQ = 2048           # queries per core
QB = 512            # query block (PSUM bank free size)
MT = 128            # m (key) tile
N_MT = N // MT      # 32
N_QB = NQ // QB     # 4
GROUPS = [(0, 3), (3, 3), (6, 3), (9, 3), (12, 3), (15, 3),
          (18, 3), (21, 3), (24, 3), (27, 3), (30, 2)]

# packed fp32 input layout: [128, XW_COLS]
XK_OFS = 0                  # x full        [128, 4096]
XQ_OFS = N                  # x query half  [128, 2048]
W1T_OFS = XQ_OFS + NQ       # W1.T          [128, 128]
W2T_OFS = W1T_OFS + C       # W2.T          [128, 128]
B1_OFS = W2T_OFS + C        # b1 column     [128, 1]
B2_OFS = B1_OFS + 1         # b2 column     [128, 1]
XW_COLS = B2_OFS + 1

F32 = mybir.dt.float32
F32R = mybir.dt.float32r
BF16 = mybir.dt.bfloat16


def build_nc():
    nc = bacc.Bacc("TRN2", target_bir_lowering=False, debug=False, num_devices=8)
    xw_ext = nc.declare_dram_parameter("xw", [C, XW_COLS], F32R, isOutput=False)
    xt_ext = nc.declare_dram_parameter("xt", [C, N], BF16, isOutput=False)
    out_ext = nc.declare_dram_parameter("out", [C, NQ], F32, isOutput=True)

    with ExitStack() as ctx:
        tc = ctx.enter_context(tile.TileContext(nc))
        consts = ctx.enter_context(tc.tile_pool(name="consts", bufs=1))
        sb_in = ctx.enter_context(tc.tile_pool(name="sb_in", bufs=1))
        sb_kq = ctx.enter_context(tc.tile_pool(name="sb_kq", bufs=1))
        sb_e = ctx.enter_context(tc.tile_pool(name="sb_e", bufs=2))
        sb_acc = ctx.enter_context(tc.tile_pool(name="sb_acc", bufs=2))
        sb_tail = ctx.enter_context(tc.tile_pool(name="sb_tail", bufs=2))
        ps_s = ctx.enter_context(tc.tile_pool(name="ps_s", bufs=2, space="PSUM"))
        ps_o = ctx.enter_context(tc.tile_pool(name="ps_o", bufs=1, space="PSUM"))
        ps_r = ctx.enter_context(tc.tile_pool(name="ps_r", bufs=1, space="PSUM"))

        ones_bf = consts.tile([C, 1], BF16, tag="ones_bf")
        nc.vector.memset(ones_bf[:], 1.0)
        ones_f32 = consts.tile([1, C], F32, tag="ones_f32")
        nc.vector.memset(ones_f32[:], 1.0)
        shift = consts.tile([C, 1], F32, tag="shift")
        nc.vector.memset(shift[:], -30.0)
        # warm the exp table early (ACT_TABLE_LOAD ~2.7us)
        warm = consts.tile([1, 16], F32, tag="warm")
        nc.vector.memset(warm[:], 0.0)
        warm_o = consts.tile([1, 16], F32, tag="warm_o")
        nc.scalar.activation(warm_o[:], warm[:], mybir.ActivationFunctionType.Exp)

        xw = sb_in.tile([C, XW_COLS], F32R, tag="xw")
        nc.sync.dma_start(xw[:], xw_ext[:])
        xt = sb_in.tile([C, N], BF16, tag="xt")
        nc.sync.dma_start(xt[:], xt_ext[:])

        kt = sb_kq.tile([C, N], F32R, tag="kt")      # K = W2 x + b2
        qt = sb_kq.tile([C, NQ], F32R, tag="qt")     # Q = W1 x + b1 (query half)

        # projections (fp32r matmuls; bias added during PSUM->SBUF evac on DVE)
        for j in range(N // QB):
            ps = ps_s.tile([C, QB], F32, tag="s")
            nc.tensor.matmul(ps[:], xw[:, W2T_OFS:W2T_OFS + C],
                             xw[:, XK_OFS + j * QB:XK_OFS + (j + 1) * QB],
                             start=True, stop=True)
            nc.vector.tensor_scalar(
                out=kt[:, j * QB:(j + 1) * QB], in0=ps[:],
                scalar1=xw[:, B2_OFS:B2_OFS + 1].bitcast(F32), scalar2=None,
                op0=mybir.AluOpType.add)
        for j in range(NQ // QB):
            ps = ps_s.tile([C, QB], F32, tag="s")
            nc.tensor.matmul(ps[:], xw[:, W1T_OFS:W1T_OFS + C],
                             xw[:, XQ_OFS + j * QB:XQ_OFS + (j + 1) * QB],
                             start=True, stop=True)
            nc.vector.tensor_scalar(
                out=qt[:, j * QB:(j + 1) * QB], in0=ps[:],
                scalar1=xw[:, B1_OFS:B1_OFS + 1].bitcast(F32), scalar2=None,
                op0=mybir.AluOpType.add)

        for qb in range(N_QB):
            o_ps = ps_o.tile([C, QB], F32, tag="o")
            acc = sb_acc.tile([C, 3 * QB], BF16, tag="acc")
            for gi, (mt0, gn) in enumerate(GROUPS):
                s_ps = ps_s.tile([C, gn * QB], F32, tag="s")
                for j in range(gn):
                    mt = mt0 + j
                    nc.tensor.matmul(
                        s_ps[:, j * QB:(j + 1) * QB],
                        kt[:, mt * MT:(mt + 1) * MT],
                        qt[:, qb * QB:(qb + 1) * QB],
                        start=True, stop=True)
                # constant shift: exact for softmax (cancels in the division)
                # but keeps e^s in fp32/bf16 range for scores up to ~+118
                e_g = sb_e.tile([C, gn * QB], BF16, tag="e")
                nc.scalar.activation(e_g[:], s_ps[:],
                                     mybir.ActivationFunctionType.Exp,
                                     bias=shift[:, 0:1])
                if gi == 0:
                    nc.vector.tensor_copy(acc[:, :gn * QB], e_g[:])
                else:
                    nc.vector.tensor_tensor(acc[:, :gn * QB], acc[:, :gn * QB],
                                            e_g[:], op=mybir.AluOpType.add)
                for j in range(gn):
                    mt = mt0 + j
                    nc.tensor.matmul(
                        o_ps[:], xt[:, mt * MT:(mt + 1) * MT],
                        e_g[:, j * QB:(j + 1) * QB],
                        start=(mt == 0), stop=(mt == N_MT - 1))

            # tail: row-sum fold -> reciprocal -> broadcast -> normalize+residual+relu
            f1 = sb_tail.tile([C, QB], BF16, tag="f1")
            nc.vector.tensor_tensor(f1[:], acc[:, 0:QB], acc[:, QB:2 * QB],
                                    op=mybir.AluOpType.add)
            f2 = sb_tail.tile([C, QB], BF16, tag="f2")
            nc.vector.tensor_tensor(f2[:], f1[:], acc[:, 2 * QB:3 * QB],
                                    op=mybir.AluOpType.add)
            r_ps = ps_r.tile([C, QB], F32, tag="r")
            nc.tensor.matmul(r_ps[0:1, :], ones_bf[:, 0:1], f2[:],
                             start=True, stop=True)
            inv = sb_tail.tile([1, QB], F32, tag="inv")
            nc.vector.reciprocal(inv[:], r_ps[0:1, :])
            bc_ps = ps_r.tile([C, QB], F32, tag="r")
            nc.tensor.matmul(bc_ps[:], ones_f32[0:1, :], inv[:],
                             start=True, stop=True)
            bc = sb_tail.tile([C, QB], F32, tag="bcs")
            nc.vector.tensor_copy(bc[:], bc_ps[:])
            t2 = sb_tail.tile([C, QB], F32, tag="t2")
            nc.vector.tensor_tensor(t2[:], o_ps[:], bc[:],
                                    op=mybir.AluOpType.mult)
            t3 = sb_tail.tile([C, QB], F32, tag="t3")
            nc.vector.tensor_tensor(
                t3[:], t2[:],
                xw[:, XQ_OFS + qb * QB:XQ_OFS + (qb + 1) * QB].bitcast(F32),
                op=mybir.AluOpType.add)
            o_out = sb_tail.tile([C, QB], F32, tag="o_out")
            nc.vector.tensor_scalar_max(o_out[:], t3[:], 0.0)
            nc.sync.dma_start(out_ext[:, qb * QB:(qb + 1) * QB], o_out[:])

    nc.compile()
    return nc


_NC_CACHE = None


def _get_nc():
    global _NC_CACHE
    if _NC_CACHE is None:
        _NC_CACHE = build_nc()
    return _NC_CACHE


def make_in_maps(x, W1, b1, W2, b2):
    x = np.asarray(x, np.float32)
    W1 = np.asarray(W1, np.float32)
    b1 = np.asarray(b1, np.float32)
    W2 = np.asarray(W2, np.float32)
    b2 = np.asarray(b2, np.float32)
    in_maps = []
    for core in range(8):
        b, h = divmod(core, 2)
        xb = x[b]                                    # [128, 4096]
        xw = np.empty((C, XW_COLS), np.float32)
        xw[:, XK_OFS:XK_OFS + N] = xb
        xw[:, XQ_OFS:XQ_OFS + NQ] = xb[:, h * NQ:(h + 1) * NQ]
        xw[:, W1T_OFS:W1T_OFS + C] = W1.T
        xw[:, W2T_OFS:W2T_OFS + C] = W2.T
        xw[:, B1_OFS] = b1
        xw[:, B2_OFS] = b2
        # xt[:, mt*128 + c] = x[b].T[mt*128 + (partition), c]
        xt = np.ascontiguousarray(
            xb.T.reshape(N_MT, MT, C).transpose(1, 0, 2).reshape(MT, N_MT * C)
        ).astype(ml_dtypes.bfloat16)
        in_maps.append({"xw": xw, "xt": xt})
    return in_maps


def run(x, W1, b1, W2, b2, trace=False):
    nc = _get_nc()
    in_maps = make_in_maps(x, W1, b1, W2, b2)
    res = run_bass_kernel_spmd(nc, in_maps, core_ids=list(range(8)), trace=trace)
    out = np.empty((B, C, N), np.float32)
    for core in range(8):
        b, h = divmod(core, 2)
        out[b][:, h * NQ:(h + 1) * NQ] = res.results[core]["out"]
    return out, res


def kernel(x, W1, b1, W2, b2):
    out, _ = run(x, W1, b1, W2, b2, trace=False)
    return out


# revision 9
# speedup vs baseline: 1.0462x; 1.0462x over previous
"""Trainium2 Bass kernel for nn_Attention_4063039062503.

Reference (per batch b, C=128 channels, N=4096 points):
    q = W1 @ x + b1;  k = W2 @ x + b2          # [C, N]
    s[n, m] = q[:, n] . k[:, m]                # [N, N]
    a = softmax(s, axis=m)
    out = relu(x + x @ a.T)                    # out[:, n] = x @ a[n, :]

Sharding: 8 cores, core i -> batch i//2, query half i%2 (2048 queries),
full 4096 keys local (no collectives).

Per-core plan (flash-attention style, scores never hit DRAM):
  - projections on TensorE in fp32r (measured ~1.6e-4 rel err, full speed)
  - S^T tiles [m=128, q=512] in PSUM via fp32r matmuls (K tile stationary)
  - exp on ScalarE PSUM->SBUF, bf16 out, groups of 3 m-tiles per call
  - O[c, q] += xT[m-tile] @ E on TensorE in bf16, accumulated in one PSUM bank
  - row-sums: DVE bf16 accumulate across groups + one ones-matmul
  - normalize via fp32 reciprocal + fp32 broadcast-matmul, then
    residual add + relu on DVE, DMA out
"""
import os
from contextlib import ExitStack

import numpy as np
import ml_dtypes

import concourse.bass as bass
import concourse.tile as tile
from concourse import bacc, mybir
from concourse.bass_utils import run_bass_kernel_spmd

B = 4
C = 128
N = 4096            # keys per batch
NQ = 2048           # queries per core
QB = 512            # query block (PSUM bank free size)
MT = 128            # m (key) tile
N_MT = N // MT      # 32
N_QB = NQ // QB     # 4

# packed fp32 input layout: [128, XW_COLS]
XK_OFS = 0                  # x full        [128, 4096]
XQ_OFS = N                  # x query half  [128, 2048]
W1T_OFS = XQ_OFS + NQ       # W1.T          [128, 128]
W2T_OFS = W1T_OFS + C       # W2.T          [128, 128]
B1_OFS = W2T_OFS + C        # b1 column     [128, 1]
B2_OFS = B1_OFS + 1         # b2 column     [128, 1]
XW_COLS = B2_OFS + 1

F32 = mybir.dt.float32
F32R = mybir.dt.float32r
BF16 = mybir.dt.bfloat16


def build_nc():
    nc = bacc.Bacc("TRN2", target_bir_lowering=False, debug=False, num_devices=8)
    xw_ext = nc.declare_dram_parameter("xw", [C, XW_COLS], F32R, isOutput=False)
    xt_ext = nc.declare_dram_parameter("xt", [C, N], BF16, isOutput=False)
    out_ext = nc.declare_dram_parameter("out", [C, NQ], F32, isOutput=True)

    with ExitStack() as ctx:
        tc = ctx.enter_context(tile.TileContext(nc))
        consts = ctx.enter_context(tc.tile_pool(name="consts", bufs=1))
        sb_in = ctx.enter_context(tc.tile_pool(name="sb_in", bufs=1))
        sb_kq = ctx.enter_context(tc.tile_pool(name="sb_kq", bufs=1))
        sb_e = ctx.enter_context(tc.tile_pool(name="sb_e", bufs=2))
        sb_acc = ctx.enter_context(tc.tile_pool(name="sb_acc", bufs=2))
        sb_tail = ctx.enter_context(tc.tile_pool(name="sb_tail", bufs=2))
        ps_s = ctx.enter_context(tc.tile_pool(name="ps_s", bufs=2, space="PSUM"))
        ps_o = ctx.enter_context(tc.tile_pool(name="ps_o", bufs=2, space="PSUM"))
        ps_r = ctx.enter_context(tc.tile_pool(name="ps_r", bufs=2, space="PSUM"))

        ones_bf = consts.tile([C, 1], BF16, tag="ones_bf")
        nc.vector.memset(ones_bf[:], 1.0)
        ones_f32 = consts.tile([1, C], F32, tag="ones_f32")
        nc.vector.memset(ones_f32[:], 1.0)
        shift = consts.tile([C, 1], F32, tag="shift")
        nc.vector.memset(shift[:], -30.0)
        # warm the exp table early (ACT_TABLE_LOAD ~2.7us)
        warm = consts.tile([1, 16], F32, tag="warm")
        nc.vector.memset(warm[:], 0.0)
        warm_o = consts.tile([1, 16], F32, tag="warm_o")
        nc.scalar.activation(warm_o[:], warm[:], mybir.ActivationFunctionType.Exp)

        xw = sb_in.tile([C, XW_COLS], F32R, tag="xw")
        nc.sync.dma_start(xw[:], xw_ext[:])
        xt = sb_in.tile([C, N], BF16, tag="xt")
        nc.sync.dma_start(xt[:], xt_ext[:])

        kt = sb_kq.tile([C, N], F32R, tag="kt")      # K = W2 x + b2
        qt = sb_kq.tile([C, NQ], F32R, tag="qt")     # Q = W1 x + b1 (query half)

        # projections (fp32r matmuls; bias added during PSUM->SBUF evac on DVE)
        for j in range(N // QB):
            ps = ps_s.tile([C, QB], F32, tag="s")
            nc.tensor.matmul(ps[:], xw[:, W2T_OFS:W2T_OFS + C],
                             xw[:, XK_OFS + j * QB:XK_OFS + (j + 1) * QB],
                             start=True, stop=True)
            nc.vector.tensor_scalar(
                out=kt[:, j * QB:(j + 1) * QB], in0=ps[:],
                scalar1=xw[:, B2_OFS:B2_OFS + 1].bitcast(F32), scalar2=None,
                op0=mybir.AluOpType.add)
        for j in range(NQ // QB):
            ps = ps_s.tile([C, QB], F32, tag="s")
            nc.tensor.matmul(ps[:], xw[:, W1T_OFS:W1T_OFS + C],
                             xw[:, XQ_OFS + j * QB:XQ_OFS + (j + 1) * QB],
                             start=True, stop=True)
            nc.vector.tensor_scalar(
                out=qt[:, j * QB:(j + 1) * QB], in0=ps[:],
                scalar1=xw[:, B1_OFS:B1_OFS + 1].bitcast(F32), scalar2=None,
                op0=mybir.AluOpType.add)

        # two passes, each covering a pair of query blocks (2*QB = 1024 q)
        for p in range(N_QB // 2):
            q0 = 2 * p * QB                      # col offset of this q-pair
            o_psA = ps_o.tile([C, QB], F32, tag="o")
            o_psB = ps_o.tile([C, QB], F32, tag="o")
            acc = sb_acc.tile([C, 2 * QB], BF16, tag="acc")
            for mt in range(N_MT):
                s_ps = ps_s.tile([C, 2 * QB], F32, tag="s")
                for j in range(2):
                    nc.tensor.matmul(
                        s_ps[:, j * QB:(j + 1) * QB],
                        kt[:, mt * MT:(mt + 1) * MT],
                        qt[:, q0 + j * QB:q0 + (j + 1) * QB],
                        start=True, stop=True)
                # constant shift: exact for softmax (cancels in the division)
                # but keeps e^s in fp32/bf16 range for scores up to ~+118
                e_g = sb_e.tile([C, 2 * QB], BF16, tag="e")
                nc.scalar.activation(e_g[:], s_ps[:],
                                     mybir.ActivationFunctionType.Exp,
                                     bias=shift[:, 0:1])
                if mt == 0:
                    nc.vector.tensor_copy(acc[:], e_g[:])
                else:
                    nc.vector.tensor_tensor(acc[:], acc[:], e_g[:],
                                            op=mybir.AluOpType.add)
                for j, o_ps in enumerate((o_psA, o_psB)):
                    nc.tensor.matmul(
                        o_ps[:], xt[:, mt * MT:(mt + 1) * MT],
                        e_g[:, j * QB:(j + 1) * QB],
                        start=(mt == 0), stop=(mt == N_MT - 1))

            # per-qb tail: row-sum -> reciprocal -> broadcast -> norm+residual+relu
            for j, o_ps in enumerate((o_psA, o_psB)):
                qofs = q0 + j * QB
                r_ps = ps_r.tile([C, QB], F32, tag="r")
                nc.tensor.matmul(r_ps[0:1, :], ones_bf[:, 0:1],
                                 acc[:, j * QB:(j + 1) * QB],
                                 start=True, stop=True)
                inv = sb_tail.tile([1, QB], F32, tag="inv")
                nc.vector.reciprocal_approx_fast(inv[:], r_ps[0:1, :])
                bc_ps = ps_r.tile([C, QB], F32, tag="r")
                nc.tensor.matmul(bc_ps[:], ones_f32[0:1, :], inv[:],
                                 start=True, stop=True)
                bc = sb_tail.tile([C, QB], F32, tag="bcs")
                nc.vector.tensor_copy(bc[:], bc_ps[:])
                t2 = sb_tail.tile([C, QB], F32, tag="t2")
                nc.vector.tensor_tensor(t2[:], o_ps[:], bc[:],
                                        op=mybir.AluOpType.mult)
                t3 = sb_tail.tile([C, QB], F32, tag="t3")
                nc.vector.tensor_tensor(
                    t3[:], t2[:],
                    xw[:, XQ_OFS + qofs:XQ_OFS + qofs + QB].bitcast(F32),
                    op=mybir.AluOpType.add)
                o_out = sb_tail.tile([C, QB], F32, tag="o_out")
                nc.vector.tensor_scalar_max(o_out[:], t3[:], 0.0)
                nc.sync.dma_start(out_ext[:, qofs:qofs + QB], o_out[:])

    nc.compile()
    return nc


_NC_CACHE = None


def _get_nc():
    global _NC_CACHE
    if _NC_CACHE is None:
        _NC_CACHE = build_nc()
    return _NC_CACHE


def make_in_maps(x, W1, b1, W2, b2):
    x = np.asarray(x, np.float32)
    W1 = np.asarray(W1, np.float32)
    b1 = np.asarray(b1, np.float32)
    W2 = np.asarray(W2, np.float32)
    b2 = np.asarray(b2, np.float32)
    in_maps = []
    for core in range(8):
        b, h = divmod(core, 2)
        xb = x[b]                                    # [128, 4096]
        xw = np.empty((C, XW_COLS), np.float32)
        xw[:, XK_OFS:XK_OFS + N] = xb
        xw[:, XQ_OFS:XQ_OFS + NQ] = xb[:, h * NQ:(h + 1) * NQ]
        xw[:, W1T_OFS:W1T_OFS + C] = W1.T
        xw[:, W2T_OFS:W2T_OFS + C] = W2.T
        xw[:, B1_OFS] = b1
        xw[:, B2_OFS] = b2
        # xt[:, mt*128 + c] = x[b].T[mt*128 + (partition), c]
        xt = np.ascontiguousarray(
            xb.T.reshape(N_MT, MT, C).transpose(1, 0, 2).reshape(MT, N_MT * C)
        ).astype(ml_dtypes.bfloat16)
        in_maps.append({"xw": xw, "xt": xt})
    return in_maps


def run(x, W1, b1, W2, b2, trace=False):
    nc = _get_nc()
    in_maps = make_in_maps(x, W1, b1, W2, b2)
    res = run_bass_kernel_spmd(nc, in_maps, core_ids=list(range(8)), trace=trace)
    out = np.empty((B, C, N), np.float32)
    for core in range(8):
        b, h = divmod(core, 2)
        out[b][:, h * NQ:(h + 1) * NQ] = res.results[core]["out"]
    return out, res


def kernel(x, W1, b1, W2, b2):
    out, _ = run(x, W1, b1, W2, b2, trace=False)
    return out
